# revision 23
# baseline (speedup 1.0000x reference)
"""Trainium2 Bass kernel for nn_CyclicAttention (dense transformer attention layer).

Full computation:
    Q = q @ Wq + bq ; K = k @ Wk + bk ; V = v @ Wv + bv          [B,S,H]
    per head (DK=64): scores = Q K^T / 8 ; P = softmax(scores)
    attn = P V ; merged = concat heads                            [B,S,H]
    h = merged @ Wo + bo ; c = merged @ Wc + bc
    returns (c, h)

Sharding: 2D tensor-parallel — core (g, s) with g in 0..3 (head group: 4
heads = 256 projection columns) and s in 0..1 (batch). Each core reads only
its batch's host-transposed activations qT/kT/vT [H, S], its 256-column
slices of Wq/Wk/Wv and 256-row slices of Wo/Wc, and computes:
    Q.T/K.T [cols, tokens] (two 128-row column-halves),
    V in natural [tokens, cols] per head (via PE transpose),
    scores-transposed attention per head pair (PE-row packed):
        S.T[k,q] = K.T^T @ Q.T ; P.T = exp(S.T / 8)  (no max subtraction)
        attn.T (+ denominator row) = [V | 1]^T @ P.T  (ones-column trick)
        merged.T = attn.T * (1/denom)  (partition-broadcast reciprocal)
    h.T_partial = Wo-row-chunks^T @ merged.T  (256-contraction, 2 chunks)
Host sums the 4 head-group partials per batch, adds bo/bc, transposes.

MM_DTYPE selects the tensor-engine operand dtype: "float32" (exact, 4
cycles/row), "float32r" (TF32-like, 1 cycle/row), "bfloat16" (1 cycle/row,
half DMA traffic).
"""

import numpy as np

H = 1024
NH = 16
DK = 64
C = 1024
B = 2
S = 2048
T = B * S
NCORES = 8
NG = 4             # head groups
CPC = H // NG      # 256 cols (4 heads) per core
P = 128
TCH = 512          # matmul moving-dim chunk
NHC = H // P       # 8 contraction chunks for projections
SKT = S // P       # 16 key tiles
SQC = S // TCH     # 4 query chunks
SVT = S // P       # 16 V token tiles

MM_DTYPE = "float32r"

_CACHE = {}


def _np_io_dtype(mm_dtype):
    if mm_dtype == "bfloat16":
        import ml_dtypes
        return np.dtype(ml_dtypes.bfloat16)
    return np.dtype(np.float32)


def _build_program(loop_n=None, mm_dtype=MM_DTYPE):
    import contextlib

    import concourse.tile as tile
    from concourse import bacc, mybir
    from concourse.masks import make_identity

    fp32 = mybir.dt.float32
    mdt = getattr(mybir.dt, mm_dtype)
    Act = mybir.ActivationFunctionType

    nc = bacc.Bacc("TRN2", target_bir_lowering=False, debug=False, num_devices=NCORES)

    qT = nc.dram_tensor("qT", [H, S], mdt, kind="ExternalInput").ap()
    kT = nc.dram_tensor("kT", [H, S], mdt, kind="ExternalInput").ap()
    vT = nc.dram_tensor("vT", [H, S], mdt, kind="ExternalInput").ap()
    wq = nc.dram_tensor("wq", [H, CPC], mdt, kind="ExternalInput").ap()
    wk = nc.dram_tensor("wk", [H, CPC], mdt, kind="ExternalInput").ap()
    wv = nc.dram_tensor("wv", [H, CPC], mdt, kind="ExternalInput").ap()
    wo = nc.dram_tensor("wo", [CPC, H], mdt, kind="ExternalInput").ap()
    wc = nc.dram_tensor("wc", [CPC, C], mdt, kind="ExternalInput").ap()
    bq = nc.dram_tensor("bq", [CPC, 1], fp32, kind="ExternalInput").ap()
    bk = nc.dram_tensor("bk", [CPC, 1], fp32, kind="ExternalInput").ap()
    bv = nc.dram_tensor("bv", [1, CPC], fp32, kind="ExternalInput").ap()
    hT = nc.dram_tensor("hT", [H, S], fp32, kind="ExternalOutput").ap()
    cT = nc.dram_tensor("cT", [C, S], fp32, kind="ExternalOutput").ap()

    with tile.TileContext(nc) as tc:
        with (
            tc.tile_pool(name="const", bufs=1) as const,
            tc.tile_pool(name="wqkv", bufs=1) as wpool,
            tc.tile_pool(name="acts", bufs=1) as acts,
            tc.tile_pool(name="xin", bufs=3) as xin,
            tc.tile_pool(name="pt", bufs=8) as ptp,
            tc.tile_pool(name="small", bufs=2) as small,
            tc.tile_pool(name="ostage", bufs=3) as ostage,
            tc.tile_pool(name="ps_proj", bufs=2, space="PSUM") as ps_proj,
            tc.tile_pool(name="ps_s", bufs=3, space="PSUM") as ps_s,
            tc.tile_pool(name="ps_a", bufs=2, space="PSUM") as ps_a,
            tc.tile_pool(name="ps_o", bufs=1, space="PSUM") as ps_o,
            tc.For_i(0, loop_n, 1) if loop_n else contextlib.nullcontext(),
        ):
            # ---- constants ----
            # weight slices as [h-chunk part, h-chunk idx, col-half, 128]
            wq_sb = wpool.tile([P, NHC, 2, P], mdt, tag="wq")
            nc.sync.dma_start(wq_sb[:], wq.rearrange("(a p) (u c) -> p a u c", p=P, c=P))
            wk_sb = wpool.tile([P, NHC, 2, P], mdt, tag="wk")
            nc.sync.dma_start(wk_sb[:], wk.rearrange("(a p) (u c) -> p a u c", p=P, c=P))
            wv_sb = wpool.tile([P, NHC, 2, P], mdt, tag="wv")
            nc.sync.dma_start(wv_sb[:], wv.rearrange("(a p) (u c) -> p a u c", p=P, c=P))
            # output weights as [c-chunk part, c-chunk idx, H]
            wo_sb = wpool.tile([P, 2, H], mdt, tag="wo")
            nc.sync.dma_start(wo_sb[:], wo.rearrange("(a p) j -> p a j", p=P))
            wc_sb = wpool.tile([P, 2, C], mdt, tag="wc")
            nc.sync.dma_start(wc_sb[:], wc.rearrange("(a p) j -> p a j", p=P))
            bq_sb = const.tile([P, 2], fp32, tag="bq")
            nc.sync.dma_start(bq_sb[:], bq.rearrange("(u p) o -> p (u o)", p=P))
            bk_sb = const.tile([P, 2], fp32, tag="bk")
            nc.sync.dma_start(bk_sb[:], bk.rearrange("(u p) o -> p (u o)", p=P))
            bv_row = const.tile([1, CPC], fp32, tag="bvr")
            nc.sync.dma_start(bv_row[:], bv[:, :])
            bv_bc = const.tile([P, CPC], fp32, tag="bvb")
            nc.gpsimd.partition_broadcast(bv_bc[:], bv_row[:])
            if mdt == fp32:
                ident = const.tile([P, P], fp32, tag="ident")
                make_identity(nc, ident[:])
            else:
                ident_f = const.tile([P, P], fp32, tag="identf")
                make_identity(nc, ident_f[:])
                ident = const.tile([P, P], mdt, tag="ident")
                nc.scalar.activation(ident[:], ident_f[:], Act.Copy)
            ones_f = const.tile([P, SVT, 1], fp32, tag="onesf")
            nc.vector.memset(ones_f[:], 1.0)

            # ---- persistent activations, split per 512-token chunk so early
            # attention k-tiles don't wait on later projection chunks ----
            qTs = [[acts.tile([P, TCH], mdt, tag=f"qTs{u}_{t}", name=f"qT{u}_{t}")
                    for t in range(SQC)] for u in range(2)]
            kTs = [[acts.tile([P, TCH], mdt, tag=f"kTs{u}_{t}", name=f"kT{u}_{t}")
                    for t in range(SQC)] for u in range(2)]
            vTs = [[acts.tile([P, TCH], mdt, tag=f"vTs{u}_{t}", name=f"vT{u}_{t}")
                    for t in range(SQC)] for u in range(2)]
            # V natural per head per chunk: 65-wide tiles [t-tile -> 64 cols + one]
            vh = [[acts.tile([P, 4 * 65], mdt, tag=f"vh{h}_{t}", name=f"vh{h}_{t}")
                   for t in range(SQC)] for h in range(4)]
            mTs = [acts.tile([P, S], mdt, tag=f"mTs{u}", name=f"mT{u}") for u in range(2)]

            for h in range(4):
                for t in range(SQC):
                    nc.scalar.activation(
                        vh[h][t][:].rearrange("p (n c) -> p n c", c=65)[:, :, 64:65],
                        ones_f[:, 0:4, :], Act.Copy)

            # One x stream feeds both column-halves (2 concurrent PSUM groups).
            # Each input DMA carries 4 h-chunks (1 MB) via a 3D access pattern.
            HCG = 4
            def project_chunk(src_, w_sb, dsts, bias_sb, th):
                tw = slice(th * TCH, (th + 1) * TCH)
                src3 = src_.rearrange("(a p) t -> p a t", p=P)
                pss = [ps_proj.tile([P, TCH], fp32, tag="mm", name=f"psp{i}")
                       for i in range(2)]
                for hg in range(NHC // HCG):
                    x = xin.tile([P, HCG, TCH], mdt, tag="x")
                    nc.sync.dma_start(x[:], src3[:, hg * HCG:(hg + 1) * HCG, tw])
                    for hi in range(HCG):
                        hc = hg * HCG + hi
                        for u in range(2):
                            nc.tensor.matmul(
                                pss[u][:], lhsT=w_sb[:, hc, u, :], rhs=x[:, hi, :],
                                start=(hc == 0), stop=(hc == NHC - 1))
                for u in range(2):
                    if bias_sb is not None:
                        nc.vector.tensor_scalar_add(
                            dsts[u][th][:], pss[u][:], bias_sb[:, u:u + 1])
                    else:
                        nc.vector.tensor_copy(dsts[u][th][:], pss[u][:])

            # interleave k/v/q chunks so attention streams in early
            for th in range(SQC):
                project_chunk(kT, wk_sb, kTs, bk_sb, th)
                project_chunk(vT, wv_sb, vTs, None, th)
                # V natural tiles (PE transposes), split heads + bv
                for u in range(2):
                    for i in range(TCH // P):
                        tp = ps_proj.tile([P, TCH], fp32, tag="mm")
                        tpv = tp[:, 0:P].bitcast(mdt) if mdt != fp32 else tp[:, 0:P]
                        nc.tensor.transpose(tpv, vTs[u][th][:, i * P:(i + 1) * P], ident[:])
                        for hh in range(2):
                            h = 2 * u + hh
                            nc.vector.tensor_tensor(
                                vh[h][th][:, i * 65:i * 65 + 64],
                                tpv[:, hh * 64:(hh + 1) * 64],
                                bv_bc[:, h * 64:(h + 1) * 64],
                                op=mybir.AluOpType.add)
                project_chunk(qT, wq_sb, qTs, bq_sb, th)

            # ---- attention (qc outer, head-pair inner) + output projections ----
            for qc in range(SQC):
                qw = slice(qc * TCH, (qc + 1) * TCH)
                for u in range(2):          # head pair = column half
                    ap0 = ps_a.tile([65, TCH], fp32, tag="attn", name="ap0")
                    ap1 = ps_a.tile([65, TCH], fp32, tag="attn", name="ap1")
                    for kt in range(SKT):
                        kw = slice(kt * P, (kt + 1) * P)
                        s0 = ps_s.tile([P, TCH], fp32, tag="s", name="s0")
                        s1 = ps_s.tile([P, TCH], fp32, tag="s", name="s1")
                        kth, ki = kt // 4, kt % 4
                        kwi = slice(ki * P, (ki + 1) * P)
                        # scores.T for the pair (PE rows 0-63 / 64-127)
                        nc.tensor.matmul(s0[:], lhsT=kTs[u][kth][0:64, kwi],
                                         rhs=qTs[u][qc][0:64, :], start=True, stop=True)
                        nc.tensor.matmul(s1[:], lhsT=kTs[u][kth][64:128, kwi],
                                         rhs=qTs[u][qc][64:128, :], start=True, stop=True)
                        p0 = ptp.tile([P, TCH], mdt, tag="p0")
                        nc.scalar.activation(p0[:], s0[:], Act.Exp, scale=0.125)
                        p1 = ptp.tile([P, TCH], mdt, tag="p1")
                        nc.scalar.activation(p1[:], s1[:], Act.Exp, scale=0.125)
                        nc.tensor.matmul(ap0[:], lhsT=vh[2 * u][kth][:, ki * 65:(ki + 1) * 65],
                                         rhs=p0[:], start=(kt == 0), stop=(kt == SKT - 1))
                        nc.tensor.matmul(ap1[:], lhsT=vh[2 * u + 1][kth][:, ki * 65:(ki + 1) * 65],
                                         rhs=p1[:], start=(kt == 0), stop=(kt == SKT - 1))
                    for hh, ap in ((0, ap0), (1, ap1)):
                        rec = small.tile([1, TCH], fp32, tag="rec")
                        nc.vector.reciprocal(rec[:], ap[64:65, :])
                        rbc = small.tile([64, TCH], fp32, tag="rbc")
                        nc.gpsimd.partition_broadcast(rbc[:], rec[:])
                        nc.vector.tensor_tensor(
                            mTs[u][hh * 64:(hh + 1) * 64, qw], ap[0:64, :], rbc[:],
                            op=mybir.AluOpType.mult)
                # output projections every other query chunk (1 MB staged DMAs)
                if qc % 2 == 1:
                    half = qc // 2
                    ow = slice(half * 2 * TCH, (half + 1) * 2 * TCH)
                    for j in range(H // P):
                        for m, (w_sb, outT) in enumerate(((wo_sb, hT), (wc_sb, cT))):
                            ot = ostage.tile([P, 2 * TCH], fp32, tag="ot", name="ot")
                            for q2 in range(2):
                                qw2 = slice((qc - 1 + q2) * TCH, (qc + q2) * TCH)
                                po = ps_o.tile([P, TCH], fp32, tag="o", name="po")
                                for u in range(2):
                                    nc.tensor.matmul(
                                        po[:], lhsT=w_sb[:, u, j * P:(j + 1) * P],
                                        rhs=mTs[u][:, qw2], start=(u == 0), stop=(u == 1))
                                nc.vector.tensor_copy(ot[:, q2 * TCH:(q2 + 1) * TCH], po[:])
                            if m == 0:
                                nc.gpsimd.dma_start(outT[j * P:(j + 1) * P, ow], ot[:])
                            else:
                                nc.sync.dma_start(outT[j * P:(j + 1) * P, ow], ot[:])

    nc.compile()
    return nc


def _get_program():
    if "nc" not in _CACHE:
        _CACHE["nc"] = _build_program()
    return _CACHE["nc"]


def make_in_maps(q, k, v, Wq, bq, Wk, bk, Wv, bv, Wo, bo, Wc, bc, mm_dtype=MM_DTYPE):
    iodt = _np_io_dtype(mm_dtype)
    q = np.asarray(q, np.float32).reshape(T, H)
    k = np.asarray(k, np.float32).reshape(T, H)
    v = np.asarray(v, np.float32).reshape(T, H)
    # per-batch transposed activations [H, S]
    qTb = [np.ascontiguousarray(q[s * S:(s + 1) * S].T).astype(iodt) for s in range(B)]
    kTb = [np.ascontiguousarray(k[s * S:(s + 1) * S].T).astype(iodt) for s in range(B)]
    vTb = [np.ascontiguousarray(v[s * S:(s + 1) * S].T).astype(iodt) for s in range(B)]
    wqg, wkg, wvg, wog, wcg, bqg, bkg, bvg = [], [], [], [], [], [], [], []
    for g in range(NG):
        cs = slice(g * CPC, (g + 1) * CPC)
        wqg.append(np.ascontiguousarray(np.asarray(Wq, np.float32)[:, cs]).astype(iodt))
        wkg.append(np.ascontiguousarray(np.asarray(Wk, np.float32)[:, cs]).astype(iodt))
        wvg.append(np.ascontiguousarray(np.asarray(Wv, np.float32)[:, cs]).astype(iodt))
        wog.append(np.ascontiguousarray(np.asarray(Wo, np.float32)[cs, :]).astype(iodt))
        wcg.append(np.ascontiguousarray(np.asarray(Wc, np.float32)[cs, :]).astype(iodt))
        bqg.append(np.asarray(bq, np.float32)[cs].reshape(CPC, 1).copy())
        bkg.append(np.asarray(bk, np.float32)[cs].reshape(CPC, 1).copy())
        bvg.append(np.asarray(bv, np.float32)[cs].reshape(1, CPC).copy())

    in_maps = []
    for core in range(NCORES):
        g, s = core % NG, core // NG
        in_maps.append({
            "qT": qTb[s], "kT": kTb[s], "vT": vTb[s],
            "wq": wqg[g], "wk": wkg[g], "wv": wvg[g],
            "wo": wog[g], "wc": wcg[g],
            "bq": bqg[g], "bk": bkg[g], "bv": bvg[g],
        })
    return in_maps


def combine_outputs(results, bo, bc):
    h = np.zeros((B, S, H), np.float32)
    cc = np.zeros((B, S, C), np.float32)
    for s in range(B):
        hT_full = np.zeros((H, S), np.float64)
        cT_full = np.zeros((C, S), np.float64)
        for g in range(NG):
            core = s * NG + g
            hT_full += results[core]["hT"]
            cT_full += results[core]["cT"]
        h[s] = hT_full.T.astype(np.float32) + np.asarray(bo, np.float32)
        cc[s] = cT_full.T.astype(np.float32) + np.asarray(bc, np.float32)
    return (cc, h)


def kernel(q, k, v, Wq, bq, Wk, bk, Wv, bv, Wo, bo, Wc, bc):
    from concourse.bass_utils import run_bass_kernel_spmd

    nc = _get_program()
    in_maps = make_in_maps(q, k, v, Wq, bq, Wk, bk, Wv, bv, Wo, bo, Wc, bc)
    res = run_bass_kernel_spmd(nc, in_maps, core_ids=list(range(NCORES)))
    _CACHE["last_results"] = res
    return combine_outputs(res.results, bo, bc)
